# revision 13
# baseline (speedup 1.0000x reference)
"""Trainium2 Bass kernel for nn_EnhancedBlockGabor.

Pipeline per NeuronCore (8 cores; core k owns batch b=k//2, output-row half
h0=64*(k%2)):
  1. Build per-channel conv operand P[(dyj,dx), (h,w)] = xpad[h0+h+dyj, w+dx]
     with one 4-dim-strided DRAM->SBUF DMA per channel (64 partitions:
     dyj in [0,4) x dx in [0,16), dx=15 zero-tap pad).
  2. Depthwise Gabor conv as fp32r matmuls: K=64 (60 useful), M=32
     (24 filters + 8 zero pad), N=512; 4 accumulating dy-rounds whose
     free-dim AP shift covers dy = dyj + 4*dyr.  Separate RE and IM matmul
     groups land four channels in the four 32-row quadrants of psum_re /
     psum_im so the magnitude runs as full-128-partition DVE ops.
  3. mag = sqrt(re^2 + im^2): ACT square (PSUM->SBUF), custom DVE op
     (sq(Src0)+Src1), ACT sqrt.
  4. 1x1 conv: per channel-quad matmul y += w1_q^T @ mag accumulating in a
     resident [128, 2048] PSUM tile (16 N-tiles at 4 col-group quadrants).
  5. InstanceNorm: per-row sum/sumsq, fold 4 partition rows per output
     channel with a tiny selector matmul, pair-wise AllReduce (cores 2b,
     2b+1 hold the two halves of image b), normalize, DMA out.
Host side: zero-pad x, build filter/w1/selector operands, scatter to the 8
cores, and transpose-assemble the output (the reference's H<->W swap).
"""

import numpy as np

import concourse.bass_types as bt
import concourse.mybir as mybir
import concourse.tile as tile
from concourse import bacc

# ---------------------------------------------------------------- constants
B, C, H, W = 4, 32, 128, 128
SO, KK = 24, 15          # filters per (re,im), kernel size
NCORES = 8
HHALF = 64               # output rows per core
GJ, NROUNDS = 4, 4       # dy = dyj + 4*dyr, dyj<4, dyr<4 (dy=15 zero tap)
PK = 64                  # P partitions per channel: dyj*16 + dx (dx=15 pad)
PFREE_H = HHALF + 4 * (NROUNDS - 1)   # 76
PFREE = PFREE_H * W      # 9728
SLAB_H, SLAB_W = 80, 144
NT, NTILE = 16, 512      # 16 N-tiles of 512 px (4 rows each)
NPIX = HHALF * W
EPS = 1e-5
F32 = mybir.dt.float32
F32R = mybir.dt.float32r
BF16 = mybir.dt.bfloat16

_SQSUM = None


def _register_custom_ops():
    """Register fused DVE ops: SQSUM = in0^2 + in1, SQ1 = in0^2."""
    global _SQSUM
    if _SQSUM is not None:
        return _SQSUM
    import concourse.dve_ops as dops
    from concourse.dve_ops import DveOp
    from concourse.dve_spec import Spec, Src0, Src1, _has_src1, lower, sq
    from concourse.dve_uop import DveOpSpec

    def reg(name, spec):
        for op in dops.OPS:
            if op.name == name:
                return op
        row = dops._CUSTOM_DVE_ROW_BASE + len(dops.OPS)
        shas = {}
        for ver in ("v3", "v4"):
            tmp = DveOpSpec(name=name, opcode=row, uops=lower(spec, ver=ver),
                            rd1_en=_has_src1(spec))
            shas[ver] = tmp.sha(ver)
        op = DveOp(name, spec, subdim=False, uops_sha=shas)
        dops.OPS.append(op)
        dops.CUSTOM_DVE_SPECS[name] = spec
        dops._SUB_OPCODE_FOR_NAME[name] = row
        return op

    sqsum = reg("SQSUM_GABOR", Spec(
        body=sq(Src0) + Src1,
        reference=lambda in0, in1, s0, s1, imm2: (
            in0.astype(np.float32) ** 2 + in1).astype(np.float32),
    ))
    sq1 = reg("SQ1_GABOR", Spec(
        body=sq(Src0),
        reference=lambda in0, in1, s0, s1, imm2:
            (in0.astype(np.float32) ** 2).astype(np.float32),
    ))
    _SQSUM = (sqsum, sq1)
    return _SQSUM


# ---------------------------------------------------------------- device IR
def build_module():
    """Trace the per-core kernel; returns (nc, io_names)."""
    sqsum = _register_custom_ops()
    nc = bacc.Bacc("TRN2", target_bir_lowering=False, debug=False,
                   num_devices=NCORES)

    xslab = nc.dram_tensor("xslab", [C, SLAB_H, SLAB_W], F32R,
                           kind="ExternalInput")
    filt = nc.dram_tensor("filt", [128, 512], F32R, kind="ExternalInput")
    w1s = nc.dram_tensor("w1s", [128, 256], BF16, kind="ExternalInput")
    sel = nc.dram_tensor("sel", [128, 32], F32, kind="ExternalInput")
    selT = nc.dram_tensor("selT", [32, 128], F32, kind="ExternalInput")
    zout = nc.dram_tensor("zout", [C, HHALF, W], F32, kind="ExternalOutput")
    cc_in = nc.dram_tensor("cc_in", [32, 2], F32, kind="Internal")
    cc_out = nc.dram_tensor("cc_out", [32, 2], F32, kind="Internal")

    with tile.TileContext(nc) as tc:
        _body(nc, tc, sqsum, xslab, filt, w1s, sel, selT, zout, cc_in, cc_out)
    nc.compile()
    return nc


def _body(nc, tc, sqsum, xslab, filt, w1s, sel, selT, zout, cc_in, cc_out):
    ctx_pools = []

    def pool(**kw):
        cm = tc.tile_pool(**kw)
        p = cm.__enter__()
        ctx_pools.append(cm)
        return p

    const_pool = pool(name="consts", bufs=1)
    filt_sb = const_pool.tile([128, 512], F32R, name="filt_sb")
    w1_sb = const_pool.tile([128, 256], BF16, name="w1_sb")
    sel_sb = const_pool.tile([128, 32], F32, name="sel_sb")
    selT_sb = const_pool.tile([32, 128], F32, name="selT_sb")
    nc.sync.dma_start(out=filt_sb[:, :], in_=filt.ap())
    nc.sync.dma_start(out=w1_sb[:, :], in_=w1s.ap())
    nc.sync.dma_start(out=sel_sb[:, :], in_=sel.ap())
    nc.sync.dma_start(out=selT_sb[:, :], in_=selT.ap())

    ypool = pool(name="ypsum", bufs=1, space="PSUM")
    y_ps = ypool.tile([128, 4 * NTILE], F32, name="y_ps")

    def pool_cm(**kw):
        cm = tc.tile_pool(**kw)
        p = cm.__enter__()
        ctx_pools.append(cm)
        return p, cm

    ppool, ppool_cm = pool_cm(name="ptiles", bufs=2)
    cpool, cpool_cm = pool_cm(name="convps", bufs=2, space="PSUM")
    mpool, mpool_cm = pool_cm(name="magtiles", bufs=16)
    m2pool, m2pool_cm = pool_cm(name="mag2", bufs=2)

    slab_elems = SLAB_H * SLAB_W
    mag_tiles = {}

    for p in range(16):          # channel pairs (2*p, 2*p+1)
        pt = ppool.tile([128, PFREE], F32R, name=f"pt{p}", tag="pt")
        for half in range(2):
            c = 2 * p + half
            rb = 64 * half
            for dyj in range(GJ):
                srcap = bt.AP(
                    tensor=xslab.ap().tensor,
                    offset=c * slab_elems + dyj * SLAB_W,
                    ap=[[1, 16], [SLAB_W, PFREE_H], [1, W]],
                )
                nc.sync.dma_start(
                    out=pt[rb + 16 * dyj:rb + 16 * dyj + 16, :], in_=srcap)

        for g2 in range(8):      # groups of 2 N-tiles
            ps = cpool.tile([128, 2 * NTILE], F32, name=f"ps{p}_{g2}",
                            tag="ps")
            for dyr in range(NROUNDS):
                for tt in range(2):
                    t = 2 * g2 + tt
                    off = (4 * t + 4 * dyr) * W
                    nc.tensor.matmul(
                        out=ps[:, NTILE * tt:NTILE * (tt + 1)],
                        lhsT=filt_sb[:, 128 * dyr:128 * (dyr + 1)],
                        rhs=pt[:, off:off + NTILE],
                        start=(dyr == 0), stop=(dyr == NROUNDS - 1),
                        skip_group_check=True, tile_position=(0, 0),
                    )
            sqsum_op, sq1_op = sqsum
            imsq = m2pool.tile([64, 2 * NTILE], F32, name=f"imsq{p}_{g2}",
                               tag="imsq")
            nc.scalar.square(imsq[:, :], ps[64:128, :])
            mag2 = m2pool.tile([64, 2 * NTILE], F32, name=f"mag2_{p}_{g2}",
                               tag="mag2")
            nc.vector._custom_dve(sqsum_op, out=mag2[:, :], in0=ps[0:64, :],
                                  in1=imsq[:, :])
            if p % 2 == 0:
                mag_tiles[g2] = mpool.tile([128, 2 * NTILE], BF16,
                                           name=f"mag_{p}_{g2}", tag="mag")
            mag = mag_tiles[g2]
            nc.scalar.sqrt(mag[64 * (p % 2):64 * (p % 2) + 64, :],
                           mag2[:, :])
            if p % 2 == 1:
                pp = p // 2
                for tt in range(2):
                    t = 2 * g2 + tt
                    nc.tensor.matmul(
                        out=y_ps[32 * (t % 4):32 * (t % 4) + 32,
                                 NTILE * (t // 4):NTILE * (t // 4) + NTILE],
                        lhsT=w1_sb[:, 32 * pp:32 * pp + 32],
                        rhs=mag[:, NTILE * tt:NTILE * (tt + 1)],
                        start=(pp == 0), stop=(pp == 7),
                        skip_group_check=True,
                        tile_position=(0, 32 * (t % 4)),
                    )

    # release conv-era pools (PSUM banks + P-tile SBUF) before the tail
    for cm in (m2pool_cm, mpool_cm, cpool_cm, ppool_cm):
        ctx_pools.remove(cm)
        cm.__exit__(None, None, None)

    # ---------------- InstanceNorm tail
    tail = pool(name="tail", bufs=1)
    y_sb = tail.tile([128, 4 * NTILE], F32, name="y_sb")
    ysq = tail.tile([128, 4 * NTILE], F32, name="ysq")
    nc.vector.tensor_copy(y_sb[:, :], y_ps[:, :])

    from concourse.alu_op_type import AluOpType
    from concourse.dve_ops import TENSOR_TENSOR_REDUCE

    small = pool(name="small", bufs=1)
    s_part = small.tile([128, 1], F32, name="s_part")
    ss_part = small.tile([128, 1], F32, name="ss_part")
    nc.vector.tensor_reduce(s_part[:, :], y_sb[:, :],
                            mybir.AxisListType.X, AluOpType.add)
    nc.vector._custom_dve(TENSOR_TENSOR_REDUCE, out=ysq[:, :],
                          in0=y_sb[:, :], in1=y_sb[:, :], s0=0.0, s1=1.0,
                          accum_out=ss_part[:, :])

    stats2 = small.tile([128, 2], F32, name="stats2")
    nc.vector.tensor_copy(stats2[:, 0:1], s_part[:, :])
    nc.vector.tensor_copy(stats2[:, 1:2], ss_part[:, :])

    stpool = pool(name="stps", bufs=1, space="PSUM")
    st_ps = stpool.tile([32, 2], F32, name="st_ps")
    nc.tensor.matmul(out=st_ps[:, :], lhsT=sel_sb[:, :], rhs=stats2[:, :],
                     start=True, stop=True)
    st_sb = small.tile([32, 2], F32, name="st_sb")
    nc.vector.tensor_copy(st_sb[:, :], st_ps[:, :])

    nc.sync.dma_start(out=cc_in.ap(), in_=st_sb[:, :])
    nc.gpsimd.collective_compute(
        "AllReduce", AluOpType.add,
        replica_groups=[[0, 1], [2, 3], [4, 5], [6, 7]],
        ins=[cc_in.ap()], outs=[cc_out.ap()],
    )
    gst = small.tile([32, 2], F32, name="gst")
    nc.sync.dma_start(out=gst[:, :], in_=cc_out.ap())

    inv_n = 1.0 / (H * W)
    mean = small.tile([32, 1], F32, name="mean")
    ssn = small.tile([32, 1], F32, name="ssn")
    m2 = small.tile([32, 1], F32, name="m2")
    var = small.tile([32, 1], F32, name="var")
    sd = small.tile([32, 1], F32, name="sd")
    rstd = small.tile([32, 1], F32, name="rstd")
    mb = small.tile([32, 1], F32, name="mb")
    nbias = small.tile([32, 1], F32, name="nbias")
    nc.scalar.mul(mean[:, :], gst[:, 0:1], inv_n)
    nc.scalar.mul(ssn[:, :], gst[:, 1:2], inv_n)
    nc.vector.tensor_tensor(m2[:, :], mean[:, :], mean[:, :], AluOpType.mult)
    nc.vector.tensor_tensor(var[:, :], ssn[:, :], m2[:, :], AluOpType.subtract)
    nc.vector.tensor_scalar(var[:, :], var[:, :], EPS, None, AluOpType.add)
    nc.scalar.activation(sd[:, :], var[:, :],
                         mybir.ActivationFunctionType.Sqrt)
    nc.vector.reciprocal(rstd[:, :], sd[:, :])
    nc.vector.tensor_tensor(mb[:, :], mean[:, :], rstd[:, :], AluOpType.mult)
    nc.scalar.mul(nbias[:, :], mb[:, :], -1.0)

    sb2 = small.tile([32, 2], F32, name="sb2")
    nc.vector.tensor_copy(sb2[:, 0:1], rstd[:, :])
    nc.vector.tensor_copy(sb2[:, 1:2], nbias[:, :])
    bc_ps = stpool.tile([128, 2], F32, name="bc_ps")
    nc.tensor.matmul(out=bc_ps[:, :], lhsT=selT_sb[:, :], rhs=sb2[:, :],
                     start=True, stop=True)
    bc_sb = small.tile([128, 2], F32, name="bc_sb")
    nc.vector.tensor_copy(bc_sb[:, :], bc_ps[:, :])

    yn = tail.tile([128, 4 * NTILE], F32, name="yn")
    nc.scalar.activation(yn[:, :], y_sb[:, :],
                         mybir.ActivationFunctionType.Identity,
                         bias=bc_sb[:, 1:2], scale=bc_sb[:, 0:1])

    # zout[o, h, w], h = 16*blk + 4*cg + d  (row 32*cg+o, free 512*blk+128*d+w)
    for cg in range(4):
        dst = bt.AP(
            tensor=zout.ap().tensor,
            offset=4 * cg * W,
            ap=[[HHALF * W, 32], [16 * W, 4], [W, 4], [1, W]],
        )
        nc.sync.dma_start(out=dst, in_=yn[32 * cg:32 * cg + 32, :])

    for cm in reversed(ctx_pools):
        cm.__exit__(None, None, None)


# ---------------------------------------------------------------- host side
def host_inputs(x, filters, w1):
    """Build the 8 per-core input dicts."""
    x = np.ascontiguousarray(x, dtype=np.float32)
    filters = np.asarray(filters, dtype=np.float32)
    w1 = np.asarray(w1, dtype=np.float32)

    xpad = np.zeros((B, C, SLAB_W, SLAB_W), np.float32)
    xpad[:, :, 7:7 + H, 7:7 + W] = x

    import ml_dtypes
    filt_h = np.zeros((128, 512), np.float32)
    for dyr in range(NROUNDS):
        for j in range(128):
            blk, so = j // 32, j % 32
            half, ri = blk % 2, blk // 2
            if so >= SO:
                continue
            rbase = 64 * half
            for dyj in range(GJ):
                dy = dyj + 4 * dyr
                if dy >= KK:
                    continue
                for dx in range(KK):
                    filt_h[rbase + dyj * 16 + dx, 128 * dyr + j] = \
                        filters[so, ri, dy, dx]

    w1_h = np.zeros((128, 256), np.float32)
    for pp in range(8):
        for r in range(128):
            so = r % 32
            if so >= SO:
                continue
            ch = 4 * pp + 2 * (r // 64) + (r % 64) // 32
            w1_h[r, 32 * pp:32 * pp + 32] = w1[:, ch * SO + so]
    w1_h = w1_h.astype(ml_dtypes.bfloat16)
    sel_h = np.zeros((128, 32), np.float32)
    for r in range(128):
        sel_h[r, r % 32] = 1.0
    selT_h = np.ascontiguousarray(sel_h.T)

    ins = []
    for k in range(NCORES):
        b, half = k // 2, k % 2
        slab = np.ascontiguousarray(xpad[b][:, 64 * half:64 * half + SLAB_H, :])
        ins.append({"xslab": slab, "filt": filt_h, "w1s": w1_h,
                    "sel": sel_h, "selT": selT_h})
    return ins


def assemble(results):
    out = np.zeros((B, C, H, W), np.float32)
    for k in range(NCORES):
        b, half = k // 2, k % 2
        z = results[k]["zout"]            # (32, 64, 128) = (o, h, w)
        out[b][:, :, 64 * half:64 * half + HHALF] = z.transpose(0, 2, 1)
    return out


_NC_CACHE = None


def _get_module():
    global _NC_CACHE
    if _NC_CACHE is None:
        _NC_CACHE = build_module()
    return _NC_CACHE


def kernel(x, filters, w1, b1, **run_kwargs):
    from concourse.bass_utils import run_bass_kernel_spmd
    nc = _get_module()
    ins = host_inputs(x, filters, w1)
    res = run_bass_kernel_spmd(nc, ins, core_ids=list(range(NCORES)),
                               **run_kwargs)
    out = assemble(res.results)
    kernel.last_result = res
    return out
